# revision 9
# baseline (speedup 1.0000x reference)
"""PointGNN Bass kernel for 8 trn2 cores.

Sharding: core c = 4*b + s handles batch b, source points i in [128s, 128s+128).
Per layer: MLP_h locally; edge MLP via K-padded neighbor gathers from a
per-batch XS table in DRAM (row j = [x_j(3), s_j(8), 1, pad]); e_in(i,j) =
[p_i + x_j, s_j, 1] with p_i = delta_i - x_i; max-pool over K; MLP_g +
residual; AllGather state slices within each batch group of 4 cores.
All LN gammas are folded into the next linear's weights on the host
(exact since betas are zero); f1 bias enters via the ones column.
"""
import os
import numpy as np

import concourse.bacc as bacc
import concourse.bass as bass
import concourse.tile as tile
from concourse import mybir
from concourse import bass_utils
from concourse.masks import make_identity

F32 = mybir.dt.float32
I32 = mybir.dt.int32
AF = mybir.ActivationFunctionType
ALU = mybir.AluOpType

B, N, SD = 2, 512, 8
L = 3
R = 0.05
EPS = 1e-5
NCORES = 8
NI = 128  # i's per core

# optional NTFF profiling shim (only needed when tracing)
_LAST_EXEC_NS = None


def _install_trace_shim():
    try:
        import sys
        import types
        import antenv
        if "antenv.axon_hooks" in sys.modules:
            return True
        m = types.ModuleType("antenv.axon_hooks")
        m._hook = None
        m.set_axon_ntff_profile_hook = lambda h: setattr(m, "_hook", h)
        m.get_axon_ntff_profile_hook = lambda: m._hook
        sys.modules["antenv.axon_hooks"] = m
        antenv.axon_hooks = m
        from trn_agent_boot.trn_boot import _ntff_profile_via_ctypes
        m.set_axon_ntff_profile_hook(
            _ntff_profile_via_ctypes("/opt/axon/libaxon_pjrt.so")
        )
        return True
    except Exception:
        return False


def _lin_block(nc, sb, ps, ident, act_ap, kin, m, w_ap, ln, relu, scale_ap=None,
               eps_ap=None):
    """slot-major linear (+optional LN, ReLU/mask). act_ap: SBUF (128, kin)."""
    tp = ps.tile([kin, 128], F32)
    nc.tensor.transpose(out=tp[:], in_=act_ap, identity=ident)
    aT = sb.tile([kin, 128], F32)
    nc.scalar.copy(out=aT[:], in_=tp[:])
    pm = ps.tile([128, m], F32)
    nc.tensor.matmul(out=pm[:], lhsT=aT[:], rhs=w_ap, start=True, stop=True)
    out = sb.tile([128, m], F32)
    if ln:
        stats = sb.tile([128, 6], F32)
        mv = sb.tile([128, 2], F32)
        nc.vector.bn_stats(out=stats[:], in_=pm[:])
        nc.vector.bn_aggr(out=mv[:], in_=stats[:])
        rstd = sb.tile([128, 1], F32)
        nc.scalar.activation(out=rstd[:], in_=mv[:, 1:2], func=AF.Sqrt,
                             bias=eps_ap)
        nc.vector.reciprocal(out=rstd[:], in_=rstd[:])
        nmr = sb.tile([128, 1], F32)
        nc.vector.tensor_tensor(out=nmr[:], in0=mv[:, 0:1], in1=rstd[:], op=ALU.mult)
        nc.vector.tensor_scalar_mul(nmr[:], nmr[:], -1.0)
        nc.scalar.activation(out=out[:], in_=pm[:], func=AF.Relu,
                             scale=rstd[:], bias=nmr[:])
    elif relu:
        if scale_ap is not None:
            nc.scalar.activation(out=out[:], in_=pm[:], func=AF.Relu, scale=scale_ap)
        else:
            nc.scalar.activation(out=out[:], in_=pm[:], func=AF.Relu)
    else:
        nc.vector.tensor_copy(out=out[:], in_=pm[:])
    return out


_BUILD_CACHE = {}


def _build(K):
    if K in _BUILD_CACHE:
        return _BUILD_CACHE[K]
    nc = bacc.Bacc("TRN2", target_bir_lowering=False, debug=False,
                   num_devices=NCORES)

    s0 = nc.dram_tensor("s0", [NI, SD], F32, kind="ExternalInput")
    xs0 = nc.dram_tensor("xs0", [N, 16], F32, kind="ExternalInput")
    xo = nc.dram_tensor("xo", [NI, 3], F32, kind="ExternalInput")
    idxs = nc.dram_tensor("idxs", [NI, K], I32, kind="ExternalInput")
    maskd = nc.dram_tensor("maskd", [NI, K], F32, kind="ExternalInput")
    wspec = dict(h1=(8, 64), h2=(64, 128), h3=(128, 3),
                 f1=(12, 64), f2=(64, 128), f3=(128, 128),
                 g1=(128, 64), g2=(64, 32), g3=(32, 8))
    wd = {k: nc.dram_tensor(k, [L, ki, m], F32, kind="ExternalInput")
          for k, (ki, m) in wspec.items()}
    o_state = nc.dram_tensor("o_state", [NI, SD], F32, kind="ExternalOutput")

    with tile.TileContext(nc) as tc:
        with (
            tc.tile_pool(name="const", bufs=1) as cp,
            tc.tile_pool(name="sb", bufs=3) as sb,
            tc.tile_pool(name="ps", bufs=2, space="PSUM") as ps,
            tc.tile_pool(name="dram", bufs=1, space="DRAM") as dr,
        ):
            ident_t = cp.tile([128, 128], F32)
            make_identity(nc, ident_t[:])
            ident = ident_t[:]
            eps_t = cp.tile([128, 1], F32)
            nc.vector.memset(eps_t[:], EPS)
            eps = eps_t[:]
            idx_sb = cp.tile([NI, K], I32)
            nc.sync.dma_start(out=idx_sb[:], in_=idxs[:])
            mask_sb = cp.tile([NI, K], F32)
            nc.sync.dma_start(out=mask_sb[:], in_=maskd[:])
            xo_sb = cp.tile([NI, 3], F32)
            nc.sync.dma_start(out=xo_sb[:], in_=xo[:])
            W = {}
            for k, (ki, m) in wspec.items():
                for t in range(L):
                    wt = cp.tile([ki, m], F32, name=f"w_{k}_{t}")
                    nc.sync.dma_start(out=wt[:], in_=wd[k][t])
                    W[(k, t)] = wt

            xs_work = dr.tile([N, 16], F32)
            nc.sync.dma_start(out=xs_work[:], in_=xs0[:])
            ag_in = [dr.tile([NI, SD], F32, name=f"ag_in{t}")
                     for t in range(L - 1)]
            ag_out = [dr.tile([N, SD], F32, name=f"ag_out{t}")
                      for t in range(L - 1)]

            s_cur = cp.tile([NI, SD], F32)
            nc.sync.dma_start(out=s_cur[:], in_=s0[:])

            for t in range(L):
                # MLP_h -> delta (128, 3)
                a = _lin_block(nc, sb, ps, ident, s_cur[:], 8, 64,
                               W[("h1", t)][:], True, True, eps_ap=eps)
                a = _lin_block(nc, sb, ps, ident, a[:], 64, 128,
                               W[("h2", t)][:], True, True, eps_ap=eps)
                delta = _lin_block(nc, sb, ps, ident, a[:], 128, 3,
                                   W[("h3", t)][:], False, False)
                ptile = sb.tile([NI, 16], F32)
                nc.vector.memset(ptile[:], 0.0)
                nc.vector.tensor_tensor(out=ptile[:, 0:3], in0=delta[:, 0:3],
                                        in1=xo_sb[:], op=ALU.subtract)

                pool = sb.tile([NI, 128], F32)
                nc.vector.memset(pool[:], 0.0)
                for k in range(K):
                    g = sb.tile([NI, 16], F32)
                    nc.gpsimd.indirect_dma_start(
                        out=g[:], out_offset=None, in_=xs_work[:],
                        in_offset=bass.IndirectOffsetOnAxis(
                            ap=idx_sb[:, k:k + 1], axis=0),
                    )
                    e = sb.tile([NI, 16], F32)
                    nc.vector.tensor_tensor(out=e[:], in0=g[:], in1=ptile[:],
                                            op=ALU.add)
                    b1 = _lin_block(nc, sb, ps, ident, e[:, 0:12], 12, 64,
                                    W[("f1", t)][:], True, True, eps_ap=eps)
                    b2 = _lin_block(nc, sb, ps, ident, b1[:], 64, 128,
                                    W[("f2", t)][:], True, True, eps_ap=eps)
                    b3 = _lin_block(nc, sb, ps, ident, b2[:], 128, 128,
                                    W[("f3", t)][:], False, True,
                                    scale_ap=mask_sb[:, k:k + 1])
                    nc.vector.tensor_tensor(out=pool[:], in0=pool[:],
                                            in1=b3[:], op=ALU.max)

                c = _lin_block(nc, sb, ps, ident, pool[:], 128, 64,
                               W[("g1", t)][:], True, True, eps_ap=eps)
                c = _lin_block(nc, sb, ps, ident, c[:], 64, 32,
                               W[("g2", t)][:], True, True, eps_ap=eps)
                c = _lin_block(nc, sb, ps, ident, c[:], 32, 8,
                               W[("g3", t)][:], False, True)
                s_new = cp.tile([NI, SD], F32, name=f"s_new{t}")
                nc.vector.tensor_tensor(out=s_new[:], in0=c[:], in1=s_cur[:],
                                        op=ALU.add)
                if t < L - 1:
                    nc.gpsimd.dma_start(out=ag_in[t][:], in_=s_new[:])
                    nc.gpsimd.collective_compute(
                        "AllGather", ALU.bypass,
                        replica_groups=[[0, 1, 2, 3], [4, 5, 6, 7]],
                        ins=[ag_in[t].opt()], outs=[ag_out[t].opt()],
                    )
                    nc.gpsimd.dma_start(out=xs_work[:, 3:11], in_=ag_out[t][:])
                s_cur = s_new
            nc.sync.dma_start(out=o_state[:], in_=s_cur[:])
    nc.compile()
    _BUILD_CACHE[K] = nc
    return nc


def _np(a):
    return np.asarray(a)


def kernel(state, frame_sz, params):
    state = _np(state).astype(np.float32)
    frame_sz = _np(frame_sz).astype(np.int32)
    p = {k: _np(v).astype(np.float32) for k, v in params.items()}

    x = state[..., :3]
    d = x[:, :, None, :] - x[:, None, :, :]
    d2 = np.einsum("bijc,bijc->bij", d, d)
    rng = np.arange(N)
    pm = np.maximum(rng[None, :], rng[:, None])[None] < frame_sz[:, None, None]
    adj = (d2 < R) & pm
    deg = adj.sum(-1)
    K = max(int(deg.max()), 1)

    idxs = np.zeros((B, N, K), np.int32)
    mask = np.zeros((B, N, K), np.float32)
    for b in range(B):
        for i in range(N):
            js = np.nonzero(adj[b, i])[0]
            idxs[b, i, :len(js)] = js
            mask[b, i, :len(js)] = 1.0

    # fold LN gammas into the following linear (betas/biases are zero)
    def wT(w, gamma=None):
        if gamma is not None:
            w = w * gamma[None, :]
        return np.ascontiguousarray(w.T)

    wp = {k: np.zeros((L,) + s, np.float32) for k, s in
          dict(h1=(8, 64), h2=(64, 128), h3=(128, 3), f1=(12, 64),
               f2=(64, 128), f3=(128, 128), g1=(128, 64), g2=(64, 32),
               g3=(32, 8)).items()}
    for t in range(L):
        wp["h1"][t] = wT(p["h_w1"][t])
        wp["h2"][t] = wT(p["h_w2"][t], p["h_g1"][t])
        wp["h3"][t] = wT(p["h_w3"][t], p["h_g2"][t])
        wp["f1"][t, :11] = wT(p["f_w1"][t])
        wp["f1"][t, 11] = p["f_b1"][t]
        wp["f2"][t] = wT(p["f_w2"][t], p["f_g1"][t])
        wp["f3"][t] = wT(p["f_w3"][t], p["f_g2"][t])
        wp["g1"][t] = wT(p["g_w1"][t])
        wp["g2"][t] = wT(p["g_w2"][t], p["g_g1"][t])
        wp["g3"][t] = wT(p["g_w3"][t], p["g_g2"][t])

    xs0 = np.zeros((B, N, 16), np.float32)
    xs0[:, :, 0:3] = x
    xs0[:, :, 3:11] = state
    xs0[:, :, 11] = 1.0

    nc = _build(K)
    in_maps = []
    for c in range(NCORES):
        b, s = c // 4, c % 4
        sl = slice(NI * s, NI * (s + 1))
        m = dict(s0=state[b, sl], xs0=xs0[b], xo=np.ascontiguousarray(x[b, sl]),
                 idxs=idxs[b, sl], maskd=mask[b, sl])
        m.update(wp)
        in_maps.append(m)

    trace = bool(os.environ.get("BASS_TRACE"))
    if trace:
        _install_trace_shim()
    res = bass_utils.run_bass_kernel_spmd(
        nc, in_maps, core_ids=list(range(NCORES)), trace=trace)
    global _LAST_EXEC_NS
    _LAST_EXEC_NS = res.exec_time_ns

    out = np.empty((B, N, SD), np.float32)
    for c in range(NCORES):
        b, s = c // 4, c % 4
        out[b, NI * s:NI * (s + 1)] = res.results[c]["o_state"]
    return out


# revision 11
# speedup vs baseline: 1.0080x; 1.0080x over previous
"""PointGNN Bass kernel for 8 trn2 cores.

Sharding: core c = 4*b + s handles batch b, source points i in [128s, 128s+128).
Per layer: MLP_h locally; edge MLP via K-padded neighbor gathers from a
per-batch XS table in DRAM (row j = [x_j(3), s_j(8), 1, pad]); e_in(i,j) =
[p_i + x_j, s_j, 1] with p_i = delta_i - x_i; max-pool over K; MLP_g +
residual; AllGather state slices within each batch group of 4 cores.
All LN gammas are folded into the next linear's weights on the host
(exact since betas are zero); f1 bias enters via the ones column.
"""
import os
import numpy as np

import concourse.bacc as bacc
import concourse.bass as bass
import concourse.tile as tile
from concourse import mybir
from concourse import bass_utils
from concourse.masks import make_identity

F32 = mybir.dt.float32
I32 = mybir.dt.int32
AF = mybir.ActivationFunctionType
ALU = mybir.AluOpType

B, N, SD = 2, 512, 8
L = 3
R = 0.05
EPS = 1e-5
NCORES = 8
NI = 128  # i's per core

# optional NTFF profiling shim (only needed when tracing)
_LAST_EXEC_NS = None


def _install_trace_shim():
    try:
        import sys
        import types
        import antenv
        if "antenv.axon_hooks" in sys.modules:
            return True
        m = types.ModuleType("antenv.axon_hooks")
        m._hook = None
        m.set_axon_ntff_profile_hook = lambda h: setattr(m, "_hook", h)
        m.get_axon_ntff_profile_hook = lambda: m._hook
        sys.modules["antenv.axon_hooks"] = m
        antenv.axon_hooks = m
        from trn_agent_boot.trn_boot import _ntff_profile_via_ctypes
        m.set_axon_ntff_profile_hook(
            _ntff_profile_via_ctypes("/opt/axon/libaxon_pjrt.so")
        )
        return True
    except Exception:
        return False


def _lin_block(nc, sb, ps, ident, act_ap, kin, m, w_ap, ln, relu, scale_ap=None,
               eps_ap=None):
    """slot-major linear (+optional LN, ReLU/mask). act_ap: SBUF (128, kin)."""
    tp = ps.tile([kin, 128], F32)
    nc.tensor.transpose(out=tp[:], in_=act_ap, identity=ident)
    aT = sb.tile([kin, 128], F32)
    nc.vector.tensor_copy(out=aT[:], in_=tp[:])
    pm = ps.tile([128, m], F32)
    nc.tensor.matmul(out=pm[:], lhsT=aT[:], rhs=w_ap, start=True, stop=True)
    out = sb.tile([128, m], F32)
    if ln:
        stats = sb.tile([128, 6], F32)
        mv = sb.tile([128, 2], F32)
        nc.vector.bn_stats(out=stats[:], in_=pm[:])
        nc.vector.bn_aggr(out=mv[:], in_=stats[:])
        rstd = sb.tile([128, 1], F32)
        nc.scalar.activation(out=rstd[:], in_=mv[:, 1:2], func=AF.Sqrt,
                             bias=eps_ap)
        nc.vector.reciprocal(out=rstd[:], in_=rstd[:])
        nmr = sb.tile([128, 1], F32)
        nc.vector.tensor_tensor(out=nmr[:], in0=mv[:, 0:1], in1=rstd[:], op=ALU.mult)
        nc.vector.tensor_scalar_mul(nmr[:], nmr[:], -1.0)
        nc.scalar.activation(out=out[:], in_=pm[:], func=AF.Relu,
                             scale=rstd[:], bias=nmr[:])
    elif relu:
        if scale_ap is not None:
            nc.scalar.activation(out=out[:], in_=pm[:], func=AF.Relu, scale=scale_ap)
        else:
            nc.scalar.activation(out=out[:], in_=pm[:], func=AF.Relu)
    else:
        nc.vector.tensor_copy(out=out[:], in_=pm[:])
    return out


_BUILD_CACHE = {}


def _build(K):
    if K in _BUILD_CACHE:
        return _BUILD_CACHE[K]
    nc = bacc.Bacc("TRN2", target_bir_lowering=False, debug=False,
                   num_devices=NCORES)

    s0 = nc.dram_tensor("s0", [NI, SD], F32, kind="ExternalInput")
    xs0 = nc.dram_tensor("xs0", [N, 16], F32, kind="ExternalInput")
    xo = nc.dram_tensor("xo", [NI, 3], F32, kind="ExternalInput")
    idxs = nc.dram_tensor("idxs", [NI, K], I32, kind="ExternalInput")
    maskd = nc.dram_tensor("maskd", [NI, K], F32, kind="ExternalInput")
    wspec = dict(h1=(8, 64), h2=(64, 128), h3=(128, 3),
                 f1=(12, 64), f2=(64, 128), f3=(128, 128),
                 g1=(128, 64), g2=(64, 32), g3=(32, 8))
    wd = {k: nc.dram_tensor(k, [L, ki, m], F32, kind="ExternalInput")
          for k, (ki, m) in wspec.items()}
    o_state = nc.dram_tensor("o_state", [NI, SD], F32, kind="ExternalOutput")

    with tile.TileContext(nc) as tc:
        with (
            tc.tile_pool(name="const", bufs=1) as cp,
            tc.tile_pool(name="sb", bufs=8) as sb,
            tc.tile_pool(name="ps", bufs=4, space="PSUM") as ps,
            tc.tile_pool(name="dram", bufs=1, space="DRAM") as dr,
        ):
            ident_t = cp.tile([128, 128], F32)
            make_identity(nc, ident_t[:])
            ident = ident_t[:]
            eps_t = cp.tile([128, 1], F32)
            nc.vector.memset(eps_t[:], EPS)
            eps = eps_t[:]
            idx_sb = cp.tile([NI, K], I32)
            nc.sync.dma_start(out=idx_sb[:], in_=idxs[:])
            mask_sb = cp.tile([NI, K], F32)
            nc.sync.dma_start(out=mask_sb[:], in_=maskd[:])
            xo_sb = cp.tile([NI, 3], F32)
            nc.sync.dma_start(out=xo_sb[:], in_=xo[:])
            W = {}
            for k, (ki, m) in wspec.items():
                for t in range(L):
                    wt = cp.tile([ki, m], F32, name=f"w_{k}_{t}")
                    nc.sync.dma_start(out=wt[:], in_=wd[k][t])
                    W[(k, t)] = wt

            xs_work = dr.tile([N, 16], F32)
            nc.sync.dma_start(out=xs_work[:], in_=xs0[:])
            ag_in = [dr.tile([NI, SD], F32, name=f"ag_in{t}")
                     for t in range(L - 1)]
            ag_out = [dr.tile([N, SD], F32, name=f"ag_out{t}")
                      for t in range(L - 1)]

            s_cur = cp.tile([NI, SD], F32)
            nc.sync.dma_start(out=s_cur[:], in_=s0[:])

            for t in range(L):
                # MLP_h -> delta (128, 3)
                a = _lin_block(nc, sb, ps, ident, s_cur[:], 8, 64,
                               W[("h1", t)][:], True, True, eps_ap=eps)
                a = _lin_block(nc, sb, ps, ident, a[:], 64, 128,
                               W[("h2", t)][:], True, True, eps_ap=eps)
                delta = _lin_block(nc, sb, ps, ident, a[:], 128, 3,
                                   W[("h3", t)][:], False, False)
                ptile = sb.tile([NI, 16], F32)
                nc.vector.memset(ptile[:], 0.0)
                nc.vector.tensor_tensor(out=ptile[:, 0:3], in0=delta[:, 0:3],
                                        in1=xo_sb[:], op=ALU.subtract)

                pool = sb.tile([NI, 128], F32)
                nc.vector.memset(pool[:], 0.0)
                for k in range(K):
                    g = sb.tile([NI, 16], F32)
                    nc.gpsimd.indirect_dma_start(
                        out=g[:], out_offset=None, in_=xs_work[:],
                        in_offset=bass.IndirectOffsetOnAxis(
                            ap=idx_sb[:, k:k + 1], axis=0),
                    )
                    e = sb.tile([NI, 16], F32)
                    nc.vector.tensor_tensor(out=e[:], in0=g[:], in1=ptile[:],
                                            op=ALU.add)
                    b1 = _lin_block(nc, sb, ps, ident, e[:, 0:12], 12, 64,
                                    W[("f1", t)][:], True, True, eps_ap=eps)
                    b2 = _lin_block(nc, sb, ps, ident, b1[:], 64, 128,
                                    W[("f2", t)][:], True, True, eps_ap=eps)
                    b3 = _lin_block(nc, sb, ps, ident, b2[:], 128, 128,
                                    W[("f3", t)][:], False, True,
                                    scale_ap=mask_sb[:, k:k + 1])
                    nc.vector.tensor_tensor(out=pool[:], in0=pool[:],
                                            in1=b3[:], op=ALU.max)

                c = _lin_block(nc, sb, ps, ident, pool[:], 128, 64,
                               W[("g1", t)][:], True, True, eps_ap=eps)
                c = _lin_block(nc, sb, ps, ident, c[:], 64, 32,
                               W[("g2", t)][:], True, True, eps_ap=eps)
                c = _lin_block(nc, sb, ps, ident, c[:], 32, 8,
                               W[("g3", t)][:], False, True)
                s_new = cp.tile([NI, SD], F32, name=f"s_new{t}")
                nc.vector.tensor_tensor(out=s_new[:], in0=c[:], in1=s_cur[:],
                                        op=ALU.add)
                if t < L - 1:
                    nc.gpsimd.dma_start(out=ag_in[t][:], in_=s_new[:])
                    nc.gpsimd.collective_compute(
                        "AllGather", ALU.bypass,
                        replica_groups=[[0, 1, 2, 3], [4, 5, 6, 7]],
                        ins=[ag_in[t].opt()], outs=[ag_out[t].opt()],
                    )
                    nc.gpsimd.dma_start(out=xs_work[:, 3:11], in_=ag_out[t][:])
                s_cur = s_new
            nc.sync.dma_start(out=o_state[:], in_=s_cur[:])
    nc.compile()
    _BUILD_CACHE[K] = nc
    return nc


def _np(a):
    return np.asarray(a)


def kernel(state, frame_sz, params):
    state = _np(state).astype(np.float32)
    frame_sz = _np(frame_sz).astype(np.int32)
    p = {k: _np(v).astype(np.float32) for k, v in params.items()}

    x = state[..., :3]
    d = x[:, :, None, :] - x[:, None, :, :]
    d2 = np.einsum("bijc,bijc->bij", d, d)
    rng = np.arange(N)
    pm = np.maximum(rng[None, :], rng[:, None])[None] < frame_sz[:, None, None]
    adj = (d2 < R) & pm
    deg = adj.sum(-1)
    K = max(int(deg.max()), 1)

    idxs = np.zeros((B, N, K), np.int32)
    mask = np.zeros((B, N, K), np.float32)
    for b in range(B):
        for i in range(N):
            js = np.nonzero(adj[b, i])[0]
            idxs[b, i, :len(js)] = js
            mask[b, i, :len(js)] = 1.0

    # fold LN gammas into the following linear (betas/biases are zero)
    def wT(w, gamma=None):
        if gamma is not None:
            w = w * gamma[None, :]
        return np.ascontiguousarray(w.T)

    wp = {k: np.zeros((L,) + s, np.float32) for k, s in
          dict(h1=(8, 64), h2=(64, 128), h3=(128, 3), f1=(12, 64),
               f2=(64, 128), f3=(128, 128), g1=(128, 64), g2=(64, 32),
               g3=(32, 8)).items()}
    for t in range(L):
        wp["h1"][t] = wT(p["h_w1"][t])
        wp["h2"][t] = wT(p["h_w2"][t], p["h_g1"][t])
        wp["h3"][t] = wT(p["h_w3"][t], p["h_g2"][t])
        wp["f1"][t, :11] = wT(p["f_w1"][t])
        wp["f1"][t, 11] = p["f_b1"][t]
        wp["f2"][t] = wT(p["f_w2"][t], p["f_g1"][t])
        wp["f3"][t] = wT(p["f_w3"][t], p["f_g2"][t])
        wp["g1"][t] = wT(p["g_w1"][t])
        wp["g2"][t] = wT(p["g_w2"][t], p["g_g1"][t])
        wp["g3"][t] = wT(p["g_w3"][t], p["g_g2"][t])

    xs0 = np.zeros((B, N, 16), np.float32)
    xs0[:, :, 0:3] = x
    xs0[:, :, 3:11] = state
    xs0[:, :, 11] = 1.0

    nc = _build(K)
    in_maps = []
    for c in range(NCORES):
        b, s = c // 4, c % 4
        sl = slice(NI * s, NI * (s + 1))
        m = dict(s0=state[b, sl], xs0=xs0[b], xo=np.ascontiguousarray(x[b, sl]),
                 idxs=idxs[b, sl], maskd=mask[b, sl])
        m.update(wp)
        in_maps.append(m)

    trace = bool(os.environ.get("BASS_TRACE"))
    if trace:
        _install_trace_shim()
    res = bass_utils.run_bass_kernel_spmd(
        nc, in_maps, core_ids=list(range(NCORES)), trace=trace)
    global _LAST_EXEC_NS
    _LAST_EXEC_NS = res.exec_time_ns

    out = np.empty((B, N, SD), np.float32)
    for c in range(NCORES):
        b, s = c // 4, c % 4
        out[b, NI * s:NI * (s + 1)] = res.results[c]["o_state"]
    return out
